# revision 11
# baseline (speedup 1.0000x reference)
"""Trainium2 Bass kernel: 4096x4096 fp32 image, 7x7 valid cross-correlation + bias.

Strategy
--------
Column-shard the image across 8 NeuronCores: core m computes output columns
[512*m, 512*m+512) (core 7 padded; image columns padded to 4102 on host, so
every core sees an identical input shard = 512 columns + 6 halo columns).

On each core the conv runs on the tensor engine as banded-Toeplitz matmuls:
an output row band of M=122 rows uses K=128 input rows (M + kh-1) as the
moving operand and contracts against seven stationary matrices A_dj[128, 128],
A_dj[k, m] = w[k-m, dj] (zero outside the band / beyond column 121).  The
seven column taps dj become free-axis shifts of the moving operand
(rhs = x[:, dj:dj+512]) accumulated in one PSUM bank via start/stop.

Layout: the host prepacks each shard band-partition-major, with the banded
weight table packed into the first two band slots of the same DRAM buffer,
so ONE head DMA (128 descriptors x 3.1KB) delivers the weights plus the
first input band; the bias is baked into the program as an activation
immediate.  Matmuls run dj-major across <=4 PSUM banks so slot turnaround
never gates the PE; a few dummy warm-up matmuls keep the PE busy from kernel
entry so the HAM clock is at 2.4 GHz when the real stream starts (any PE idle
gap drops the clock back to 1.2 GHz for ~4us of re-ramp).  Loads ride the
sync HWDGE ring, stores the scalar ring; PSUM eviction (+ fused bias add,
fp32->fp16 cast) on the scalar engine.  Inputs are cast to fp16 on host (PE
runs 16-bit at full rate, PSUM accumulates fp32 -> ~3e-4 rel err).
"""

import os
import sys

import numpy as np

for _p in ("/root/.axon_site/_ro/trn_rl_repo", "/opt/trn_rl_repo"):
    if os.path.isdir(_p) and _p not in sys.path:
        sys.path.append(_p)

H = W = 4096
KH = KW = 7
OH = OW = H - KH + 1            # 4090
NCORES = 8
CW = 512                        # output columns per core
CIN = CW + KW - 1               # 518 input columns per core (incl. halo)
BAND = 128 - (KH - 1)           # 122 output rows per band
NBANDS = -(-OH // BAND)         # 34
ROWS_PAD = BAND * (NBANDS - 1) + 128    # 4154 input rows incl. zero tail
GROUP = 8                       # bands per DMA batch / PSUM-bank rotation
ABSLOTS = 2                     # leading pseudo-band slots holding the A table
ABLEN = KW * 128                # 896 fp16 elems of packed A table (< 2*CIN)

_prog = None
_prog_bias = None


def _program(bias_val=0.0):
    global _prog, _prog_bias
    if _prog is not None and _prog_bias == bias_val:
        return _prog

    from contextlib import ExitStack

    import concourse.bass as bass
    import concourse.tile as tile
    from concourse import bacc, mybir

    nc = bacc.Bacc("TRN2", target_bir_lowering=False, debug=False)
    # flat free dim: slots [0, 2*CIN) hold the packed A table (+ padding),
    # slot 2+b holds band b
    xs = nc.dram_tensor(
        "xs", [128, (NBANDS + ABSLOTS) * CIN], mybir.dt.float16,
        kind="ExternalInput",
    )
    # 128 rows per band (6 zero pad rows) so the store SBUF AP keeps a
    # power-of-2 partition count -- the DGE engine spray needs it
    yd = nc.dram_tensor(
        "yd", [128, NBANDS, CW], mybir.dt.float16, kind="ExternalOutput"
    )
    xs_ap, yd_ap = xs.ap(), yd.ap()

    with tile.TileContext(nc) as tc, ExitStack() as ctx:
        consts = ctx.enter_context(tc.tile_pool(name="consts", bufs=1))
        inp = ctx.enter_context(tc.tile_pool(name="inp", bufs=3))
        pss = ctx.enter_context(tc.tile_pool(name="pss", bufs=7, space="PSUM"))
        warm = ctx.enter_context(tc.tile_pool(name="warm", bufs=1, space="PSUM"))
        outp = ctx.enter_context(tc.tile_pool(name="outp", bufs=3))

        # ONE head DMA: A table + first input band, 3.1KB per descriptor.
        # Extra head DMA instructions cost ~600ns each on the sync queue and
        # small descriptors crawl on the cold DMA ring, so merging is what
        # lets the stream start ~1.5us earlier.
        m0 = consts.tile([128, (ABSLOTS + 1) * CIN], mybir.dt.float16)
        nc.sync.dma_start(m0[:, :], xs_ap[:, 0 : (ABSLOTS + 1) * CIN])

        # Pre-warm the PE HAM clock gate during the load phase: dummy matmuls
        # (no data deps) keep the PE busy from kernel entry so the clock ramp
        # (half speed for the first ~4-6us of PE activity) burns down before
        # the head DMA lands.  A tiny memset unblocks the first short warm-ups
        # ~0.5us sooner; the longer moving operand memsets in parallel.
        junk = consts.tile([128, 128 + CW], mybir.dt.float16)
        nc.gpsimd.memset(junk[:, 0:128], 0)
        wps = warm.tile([128, CW], mybir.dt.float32)
        for _ in range(5):
            nc.tensor.matmul(
                wps[:, 0:128],
                junk[:, 0:128],
                junk[:, 0:128],
                start=True,
                stop=True,
            )
        nc.gpsimd.memset(junk[:, 128 : 128 + CW], 0)
        for _ in range(7):
            nc.tensor.matmul(
                wps[:, :],
                junk[:, 0:128],
                junk[:, 128 : 128 + CW],
                start=True,
                stop=True,
            )

        def a_slice(dj):
            return m0[:, dj * 128 : dj * 128 + 128]

        # small first groups so the PE starts earlier; tiny last group so the
        # final store + serial evictions don't hang off the kernel tail
        group_sizes = [1, 2, 4, 8, 8, 8, 2, 1]
        assert sum(group_sizes) == NBANDS
        b0 = 0
        for gi, g in enumerate(group_sizes):
            if gi == 0:
                xin, xoff = m0, ABSLOTS * CIN
            else:
                xin = inp.tile([128, GROUP * CIN], mybir.dt.float16, tag="xin")
                xoff = 0
                nc.sync.dma_start(
                    xin[:, : g * CIN],
                    xs_ap[:, (ABSLOTS + b0) * CIN : (ABSLOTS + b0 + g) * CIN],
                )

            yo = outp.tile([128, GROUP, CW], mybir.dt.float16, tag="yo")
            # dj-major over subgroups of <=4 bands: one LDWEIGHTS per dj per
            # subgroup, and at most 4 PSUM banks in flight (of 8) so slot
            # turnaround never gates the PE
            for s0 in range(0, g, 4):
                sg = min(4, g - s0)
                pst = [
                    pss.tile([128, CW], mybir.dt.float32, tag="ps", name=f"ps{b0}_{i}")
                    for i in range(s0, s0 + sg)
                ]
                for dj in range(KW):
                    for k, i in enumerate(range(s0, s0 + sg)):
                        nc.tensor.matmul(
                            pst[k][:, :],
                            a_slice(dj),
                            xin[:, xoff + i * CIN + dj : xoff + i * CIN + dj + CW],
                            start=(dj == 0),
                            stop=(dj == KW - 1),
                        )
                # the global last band holds only 64 real output rows
                # (4090 = 33*122 + 64); evicting/storing 64 partitions keeps
                # the power-of-2 spray and shortens the kernel tail
                for k, i in enumerate(range(s0, s0 + sg)):
                    rows = 64 if b0 + i == NBANDS - 1 else 128
                    # rows 122-127 are exact zeros (A columns >= BAND are zero)
                    nc.scalar.activation(
                        yo[:rows, i, :],
                        pst[k][:rows, :],
                        mybir.ActivationFunctionType.Identity,
                        bias=float(bias_val),
                        scale=1.0,
                    )
                # stores on the scalar HWDGE ring (q10), split into <=2-band
                # chunks issued right after their evictions, so the queue
                # never builds a multi-us backlog that would gate the tail
                for c0 in range(s0, s0 + sg, 2):
                    c1 = min(c0 + 2, s0 + sg)
                    rows = 64 if b0 + c1 == NBANDS else 128
                    nc.scalar.dma_start(
                        yd_ap[:rows, b0 + c0 : b0 + c1, :], yo[:rows, c0:c1, :]
                    )
            b0 += g

    nc.compile()
    _prog = nc
    _prog_bias = bias_val
    return nc


def _shards(x, weight, bias):
    x = np.asarray(x, dtype=np.float32)
    weight = np.asarray(weight, dtype=np.float32)
    bias = np.asarray(bias, dtype=np.float32)

    xp = np.zeros((ROWS_PAD, NCORES * CW + (KW - 1)), dtype=np.float16)
    xp[:H, :W] = x.astype(np.float16)

    wh = weight.astype(np.float16)
    abm = np.zeros((128, KW, 128), dtype=np.float16)
    idx = np.arange(BAND)
    for dj in range(KW):
        for di in range(KH):
            abm[idx + di, dj, idx] = wh[di, dj]

    s0, s1 = xp.strides
    ins = []
    for m in range(NCORES):
        core = xp[:, m * CW : m * CW + CIN]
        # xs[p, b, c] = core[BAND*b + p, c] -- overlapping-band strided view
        xb = np.lib.stride_tricks.as_strided(
            core, shape=(128, NBANDS, CIN), strides=(s0, BAND * s0, s1)
        )
        xse = np.zeros((128, (NBANDS + ABSLOTS) * CIN), dtype=np.float16)
        xse[:, :ABLEN] = abm.reshape(128, ABLEN)
        xse[:, ABSLOTS * CIN :] = xb.reshape(128, NBANDS * CIN)
        ins.append({"xs": xse})
    return ins


def _gather(results):
    y = np.empty((OH, OW), dtype=np.float32)
    for m in range(NCORES):
        c0 = m * CW
        c1 = min(c0 + CW, OW)
        # yd[r, b, c] = out[BAND*b + r, c] for r < BAND; rows >= BAND are pad
        full = (
            results[m]["yd"]
            .astype(np.float32)
            .transpose(1, 0, 2)[:, :BAND, :]
            .reshape(BAND * NBANDS, CW)
        )
        y[:, c0:c1] = full[:OH, : c1 - c0]
    return y


def kernel(x, weight, bias):
    from concourse.bass_utils import run_bass_kernel_spmd

    bias = np.asarray(bias, dtype=np.float32)
    nc = _program(float(bias[0]))
    in_maps = _shards(x, weight, bias)
    res = run_bass_kernel_spmd(nc, in_maps, core_ids=list(range(NCORES)))
    return _gather(res.results)


# revision 12
# speedup vs baseline: 1.0143x; 1.0143x over previous
"""Trainium2 Bass kernel: 4096x4096 fp32 image, 7x7 valid cross-correlation + bias.

Strategy
--------
Column-shard the image across 8 NeuronCores: core m computes output columns
[512*m, 512*m+512) (core 7 padded; image columns padded to 4102 on host, so
every core sees an identical input shard = 512 columns + 6 halo columns).

On each core the conv runs on the tensor engine as banded-Toeplitz matmuls:
an output row band of M=122 rows uses K=128 input rows (M + kh-1) as the
moving operand and contracts against seven stationary matrices A_dj[128, 128],
A_dj[k, m] = w[k-m, dj] (zero outside the band / beyond column 121).  The
seven column taps dj become free-axis shifts of the moving operand
(rhs = x[:, dj:dj+512]) accumulated in one PSUM bank via start/stop.

Layout: the host prepacks each shard band-partition-major, with the banded
weight table packed into the first two band slots of the same DRAM buffer,
so ONE head DMA (128 descriptors x 3.1KB) delivers the weights plus the
first input band; the bias is baked into the program as an activation
immediate.  Matmuls run dj-major across <=4 PSUM banks so slot turnaround
never gates the PE; a few dummy warm-up matmuls keep the PE busy from kernel
entry so the HAM clock is at 2.4 GHz when the real stream starts (any PE idle
gap drops the clock back to 1.2 GHz for ~4us of re-ramp).  Loads ride the
sync HWDGE ring, stores the scalar ring; PSUM eviction (+ fused bias add,
fp32->fp16 cast) on the scalar engine.  Inputs are cast to fp16 on host (PE
runs 16-bit at full rate, PSUM accumulates fp32 -> ~3e-4 rel err).
"""

import os
import sys

import numpy as np

for _p in ("/root/.axon_site/_ro/trn_rl_repo", "/opt/trn_rl_repo"):
    if os.path.isdir(_p) and _p not in sys.path:
        sys.path.append(_p)

H = W = 4096
KH = KW = 7
OH = OW = H - KH + 1            # 4090
NCORES = 8
CW = 512                        # output columns per core
CIN = CW + KW - 1               # 518 input columns per core (incl. halo)
BAND = 128 - (KH - 1)           # 122 output rows per band
NBANDS = -(-OH // BAND)         # 34
ROWS_PAD = BAND * (NBANDS - 1) + 128    # 4154 input rows incl. zero tail
GROUP = 8                       # bands per DMA batch / PSUM-bank rotation
ABSLOTS = 2                     # leading pseudo-band slots holding the A table
ABLEN = KW * 128                # 896 fp16 elems of packed A table (< 2*CIN)

_prog = None
_prog_bias = None


def _program(bias_val=0.0):
    global _prog, _prog_bias
    if _prog is not None and _prog_bias == bias_val:
        return _prog

    from contextlib import ExitStack

    import concourse.bass as bass
    import concourse.tile as tile
    from concourse import bacc, mybir

    nc = bacc.Bacc("TRN2", target_bir_lowering=False, debug=False)
    # flat free dim: slots [0, 2*CIN) hold the packed A table (+ padding),
    # slot 2+b holds band b
    xs = nc.dram_tensor(
        "xs", [128, (NBANDS + ABSLOTS) * CIN], mybir.dt.float16,
        kind="ExternalInput",
    )
    # 128 rows per band (6 zero pad rows) so the store SBUF AP keeps a
    # power-of-2 partition count -- the DGE engine spray needs it
    yd = nc.dram_tensor(
        "yd", [128, NBANDS, CW], mybir.dt.float16, kind="ExternalOutput"
    )
    xs_ap, yd_ap = xs.ap(), yd.ap()

    with tile.TileContext(nc) as tc, ExitStack() as ctx:
        consts = ctx.enter_context(tc.tile_pool(name="consts", bufs=1))
        inp = ctx.enter_context(tc.tile_pool(name="inp", bufs=3))
        pss = ctx.enter_context(tc.tile_pool(name="pss", bufs=7, space="PSUM"))
        warm = ctx.enter_context(tc.tile_pool(name="warm", bufs=1, space="PSUM"))
        outp = ctx.enter_context(tc.tile_pool(name="outp", bufs=3))

        # ONE head DMA: A table + first input band, 3.1KB per descriptor.
        # Extra head DMA instructions cost ~600ns each on the sync queue and
        # small descriptors crawl on the cold DMA ring, so merging is what
        # lets the stream start ~1.5us earlier.
        m0 = consts.tile([128, (ABSLOTS + 1) * CIN], mybir.dt.float16)
        nc.sync.dma_start(m0[:, :], xs_ap[:, 0 : (ABSLOTS + 1) * CIN])

        # Pre-warm the PE HAM clock gate during the load phase: dummy matmuls
        # (no data deps) keep the PE busy from kernel entry so the clock ramp
        # (half speed for the first ~4-6us of PE activity) burns down before
        # the head DMA lands.  A tiny memset unblocks the first short warm-ups
        # ~0.5us sooner; the longer moving operand memsets in parallel.
        junk = consts.tile([128, 128 + CW], mybir.dt.float16)
        nc.gpsimd.memset(junk[:, 0:128], 0)
        wps = warm.tile([128, CW], mybir.dt.float32)
        for _ in range(5):
            nc.tensor.matmul(
                wps[:, 0:128],
                junk[:, 0:128],
                junk[:, 0:128],
                start=True,
                stop=True,
            )
        nc.gpsimd.memset(junk[:, 128 : 128 + CW], 0)
        for _ in range(7):
            nc.tensor.matmul(
                wps[:, :],
                junk[:, 0:128],
                junk[:, 128 : 128 + CW],
                start=True,
                stop=True,
            )

        def a_slice(dj):
            return m0[:, dj * 128 : dj * 128 + 128]

        # small first groups so the PE starts earlier; tiny last group so the
        # final store + serial evictions don't hang off the kernel tail
        group_sizes = [1, 2, 4, 8, 8, 8, 2, 1]
        assert sum(group_sizes) == NBANDS
        b0 = 0
        for gi, g in enumerate(group_sizes):
            if gi == 0:
                xin, xoff = m0, ABSLOTS * CIN
            else:
                xin = inp.tile([128, GROUP * CIN], mybir.dt.float16, tag="xin")
                xoff = 0
                nc.sync.dma_start(
                    xin[:, : g * CIN],
                    xs_ap[:, (ABSLOTS + b0) * CIN : (ABSLOTS + b0 + g) * CIN],
                )

            yo = outp.tile([128, GROUP, CW], mybir.dt.float16, tag="yo")
            # dj-major over subgroups of <=4 bands: one LDWEIGHTS per dj per
            # subgroup, and at most 4 PSUM banks in flight (of 8) so slot
            # turnaround never gates the PE
            for s0 in range(0, g, 4):
                sg = min(4, g - s0)
                pst = [
                    pss.tile([128, CW], mybir.dt.float32, tag="ps", name=f"ps{b0}_{i}")
                    for i in range(s0, s0 + sg)
                ]
                for dj in range(KW):
                    for k, i in enumerate(range(s0, s0 + sg)):
                        nc.tensor.matmul(
                            pst[k][:, :],
                            a_slice(dj),
                            xin[:, xoff + i * CIN + dj : xoff + i * CIN + dj + CW],
                            start=(dj == 0),
                            stop=(dj == KW - 1),
                        )
                # the global last band holds only 64 real output rows
                # (4090 = 33*122 + 64); evicting/storing 64 partitions keeps
                # the power-of-2 spray and shortens the kernel tail
                for k, i in enumerate(range(s0, s0 + sg)):
                    rows = 64 if b0 + i == NBANDS - 1 else 128
                    # rows 122-127 are exact zeros (A columns >= BAND are zero)
                    nc.scalar.activation(
                        yo[:rows, i, :],
                        pst[k][:rows, :],
                        mybir.ActivationFunctionType.Identity,
                        bias=float(bias_val),
                        scale=1.0,
                    )
                # stores on the scalar HWDGE ring (q10), split into <=2-band
                # chunks issued right after their evictions, so the queue
                # never builds a multi-us backlog that would gate the tail
                for c0 in range(s0, s0 + sg, 2):
                    c1 = min(c0 + 2, s0 + sg)
                    rows = 64 if b0 + c1 == NBANDS else 128
                    # final store rides the (idle) sync queue so it doesn't
                    # wait behind the previous pair draining on q10
                    st = nc.sync if b0 + c1 == NBANDS else nc.scalar
                    st.dma_start(
                        yd_ap[:rows, b0 + c0 : b0 + c1, :], yo[:rows, c0:c1, :]
                    )
            b0 += g

        # Hold the PE clock at full speed through the eviction/store tail:
        # the HAM drops to half clock ~3us after the PE idles, which would
        # run the teardown barrier at half rate.
        for _ in range(10):
            nc.tensor.matmul(
                wps[:, :],
                junk[:, 0:128],
                junk[:, 128 : 128 + CW],
                start=True,
                stop=True,
            )

    nc.compile()
    _prog = nc
    _prog_bias = bias_val
    return nc


def _shards(x, weight, bias):
    x = np.asarray(x, dtype=np.float32)
    weight = np.asarray(weight, dtype=np.float32)
    bias = np.asarray(bias, dtype=np.float32)

    xp = np.zeros((ROWS_PAD, NCORES * CW + (KW - 1)), dtype=np.float16)
    xp[:H, :W] = x.astype(np.float16)

    wh = weight.astype(np.float16)
    abm = np.zeros((128, KW, 128), dtype=np.float16)
    idx = np.arange(BAND)
    for dj in range(KW):
        for di in range(KH):
            abm[idx + di, dj, idx] = wh[di, dj]

    s0, s1 = xp.strides
    ins = []
    for m in range(NCORES):
        core = xp[:, m * CW : m * CW + CIN]
        # xs[p, b, c] = core[BAND*b + p, c] -- overlapping-band strided view
        xb = np.lib.stride_tricks.as_strided(
            core, shape=(128, NBANDS, CIN), strides=(s0, BAND * s0, s1)
        )
        xse = np.zeros((128, (NBANDS + ABSLOTS) * CIN), dtype=np.float16)
        xse[:, :ABLEN] = abm.reshape(128, ABLEN)
        xse[:, ABSLOTS * CIN :] = xb.reshape(128, NBANDS * CIN)
        ins.append({"xs": xse})
    return ins


def _gather(results):
    y = np.empty((OH, OW), dtype=np.float32)
    for m in range(NCORES):
        c0 = m * CW
        c1 = min(c0 + CW, OW)
        # yd[r, b, c] = out[BAND*b + r, c] for r < BAND; rows >= BAND are pad
        full = (
            results[m]["yd"]
            .astype(np.float32)
            .transpose(1, 0, 2)[:, :BAND, :]
            .reshape(BAND * NBANDS, CW)
        )
        y[:, c0:c1] = full[:OH, : c1 - c0]
    return y


def kernel(x, weight, bias):
    from concourse.bass_utils import run_bass_kernel_spmd

    bias = np.asarray(bias, dtype=np.float32)
    nc = _program(float(bias[0]))
    in_maps = _shards(x, weight, bias)
    res = run_bass_kernel_spmd(nc, in_maps, core_ids=list(range(NCORES)))
    return _gather(res.results)
